# revision 35
# baseline (speedup 1.0000x reference)
"""Trainium2 Bass kernel for nn_BoundaryAttention — V3 (fully fused).

Shards batch B=32 across 8 NeuronCores (4 batches per core).

Key idea: ONE pixel-stationary matmul pair per 128-pixel chunk computes, in
pixel-major layout, all linear functions of the raw pixels at once:
  out[pix, 0:64]   = pf_raw      (conv, no bias)
  out[pix, 64:68]  = scores_raw  (folded q/k projections)
  out[pix, 68]     = mu_raw      (feature-mean of pf_raw)
  out[pix, 69]     = cross       (mean(conv_b * pf_raw) term for variance)
  out[pix, 70:134] = zlin_raw    (M-tilde @ pf_raw, LN-mean-fold + W1')
This kills the separate conv, all PE transposes, and the feature-major pfb
buffer of the previous version. Biases are folded downstream:
  - score bias -> exp(bias) folded into the mask multiplier (host)
  - conv_b for attention values -> added to avg post-division (CB4)
  - conv_b/mu/variance shift -> cross column + scalar folds (EPSM)
  - M~(ao + conv_b) -> K broadcast tile added to zlin on device
LayerNorm mean-subtraction is exact via M~ = W1'(I - 11^T/64); variance uses
E[pf^2]-mu^2 (ao cross-terms negligible, validated < 8e-3 rel err).
"""
import numpy as np

B, C, H, W = 32, 256, 128, 128
N = H * W               # 16384
HD, NH, DH = 64, 4, 16
B_PER = 4               # batches per core
N_CORES = 8
NCC = 9216              # contested-pixel capacity (mask ~50% of 16384; +16 sigma pad)
NCH = NCC // 128        # 72 pixel chunks per batch
PIXC = 1536             # pixel DMA chunk columns
FW = 134                # fused output width
NG = NCH // 8           # 10 groups of 8 chunks for the z pipeline

_BUILT = None


def _build(meanb):
    import concourse.bass as bass
    import concourse.mybir as mybir
    import concourse.tile as tile
    import concourse.bacc as bacc
    import bass_rust
    from concourse.alu_op_type import AluOpType

    AF = bass_rust.ActivationFunctionType
    f32 = mybir.dt.float32
    bf16 = mybir.dt.bfloat16
    AX = bass_rust.AxisListType.X

    nc = bacc.Bacc('TRN2', target_bir_lowering=False, debug=False)

    PIX = nc.dram_tensor("PIX", [B_PER, 2, 128, NCC], bf16, kind="ExternalInput")
    FTZ = nc.dram_tensor("FTZ", [B_PER, 2, 128, FW], bf16, kind="ExternalInput")
    MASKE = nc.dram_tensor("MASKE", [B_PER, 128, NCH * 4], bf16, kind="ExternalInput")
    MHT = nc.dram_tensor("MHT", [64, 256], bf16, kind="ExternalInput")
    C0CB = nc.dram_tensor("C0CB", [64, 1], f32, kind="ExternalInput")
    MT64 = nc.dram_tensor("MT64", [64, 64], bf16, kind="ExternalInput")
    CB4 = nc.dram_tensor("CB4", [4, 64], f32, kind="ExternalInput")
    B1R = nc.dram_tensor("B1R", [128, 512], bf16, kind="ExternalInput")
    W2R = nc.dram_tensor("W2R", [128, 512], bf16, kind="ExternalInput")
    I4 = nc.dram_tensor("I4", [4, 4], f32, kind="ExternalInput")
    I128 = nc.dram_tensor("I128", [128, 128], f32, kind="ExternalInput")
    EPSM = nc.dram_tensor("EPSM", [128, 1], f32, kind="ExternalInput")
    B2C = nc.dram_tensor("B2C", [128, 1], f32, kind="ExternalInput")
    OUT = nc.dram_tensor("OUT", [B_PER, NCH, 128], f32, kind="ExternalOutput")

    with tile.TileContext(nc) as tc:
        with tc.tile_pool(name="const", bufs=1) as cpool, \
             tc.tile_pool(name="pix0", bufs=3) as pixp0, \
             tc.tile_pool(name="pix1", bufs=3) as pixp1, \
             tc.tile_pool(name="ez", bufs=2) as ezp, \
             tc.tile_pool(name="zp", bufs=3) as zp, \
             tc.tile_pool(name="st", bufs=2) as stp, \
             tc.tile_pool(name="adj", bufs=2) as adjp, \
             tc.tile_pool(name="ps_f", bufs=3, space="PSUM") as psf, \
             tc.tile_pool(name="ps_ctx", bufs=1, space="PSUM") as psctxp, \
             tc.tile_pool(name="ps_sm", bufs=1, space="PSUM") as pssm, \
             tc.tile_pool(name="ps_o", bufs=1, space="PSUM") as pso:

            # ---- constants ----
            ftz = cpool.tile([128, B_PER * 2 * FW], bf16)
            for _b in range(B_PER):
                for _k in range(2):
                    _o = (_b * 2 + _k) * FW
                    nc.sync.dma_start(ftz[:, _o:_o + FW], FTZ[_b, _k])
            EW = NCH * 4
            maske = cpool.tile([128, B_PER * EW], bf16)
            for _b in range(B_PER):
                nc.sync.dma_start(maske[:, _b * EW:(_b + 1) * EW], MASKE[_b])
            mht = cpool.tile([64, 256], bf16)
            nc.sync.dma_start(mht[:], MHT[:])
            c0cb = cpool.tile([64, 1], f32)
            nc.sync.dma_start(c0cb[:], C0CB[:])
            mt64 = cpool.tile([64, 64], bf16)
            nc.sync.dma_start(mt64[:], MT64[:])
            cb4 = cpool.tile([4, 64], f32)
            nc.sync.dma_start(cb4[:], CB4[:])
            b1r = cpool.tile([128, 512], bf16)
            nc.sync.dma_start(b1r[:], B1R[:])
            w2r = cpool.tile([128, 512], bf16)
            nc.sync.dma_start(w2r[:], W2R[:])
            i4f = cpool.tile([4, 4], f32)
            nc.sync.dma_start(i4f[:], I4[:])
            i4b = cpool.tile([4, 4], bf16)
            nc.vector.tensor_copy(i4b[:], i4f[:])
            i128f = cpool.tile([128, 128], f32)
            nc.sync.dma_start(i128f[:], I128[:])
            epsm = cpool.tile([128, 1], f32)
            nc.sync.dma_start(epsm[:], EPSM[:])
            b2c = cpool.tile([128, 1], f32)
            nc.sync.dma_start(b2c[:], B2C[:])
            ones1 = cpool.tile([1, 128], bf16)
            nc.vector.memset(ones1[:], 1.0)

            # persistent pixel-major fused outputs, triple-buffered over batches
            pnm_bufs = [cpool.tile([128, NCH * FW], bf16, name=f"pnm{i}")
                        for i in range(3)]

            drain_idx = [0]

            def drain(dst, src):
                # psum->sbuf drains: 1/3 DVE, 2/3 Act (GPSIMD cannot touch PSUM)
                k = drain_idx[0]
                drain_idx[0] += 1
                if k % 3 == 0:
                    nc.vector.tensor_copy(dst, src)
                else:
                    nc.scalar.activation(dst, src, AF.Copy, bias=0.0)

            def emit_conv(b):
                pnm = pnm_bufs[b % 3]
                pnm3 = pnm[:].rearrange("p (c e) -> p c e", e=FW)
                ftz0 = ftz[:, (b * 2) * FW:(b * 2 + 1) * FW]
                ftz1 = ftz[:, (b * 2 + 1) * FW:(b * 2 + 2) * FW]

                # ---- fused conv/scores/stats/zlin, pixel-stationary ----
                for pos in range(NCC // PIXC):
                    px0 = pixp0.tile([128, PIXC], bf16, tag="px0")
                    px1 = pixp1.tile([128, PIXC], bf16, tag="px1")
                    nc.sync.dma_start(px0[:], PIX[b, 0, :, pos * PIXC:(pos + 1) * PIXC])
                    nc.sync.dma_start(px1[:], PIX[b, 1, :, pos * PIXC:(pos + 1) * PIXC])
                    for s in range(0, PIXC // 128, 2):
                        c = pos * (PIXC // 128) + s
                        pt = psf.tile([128, 2, FW], f32, tag="pfz")
                        for k in range(2):
                            nc.tensor.matmul(pt[:, k, :],
                                             px0[:, (s + k) * 128:(s + k + 1) * 128],
                                             ftz0, start=True, stop=False)
                            nc.tensor.matmul(pt[:, k, :],
                                             px1[:, (s + k) * 128:(s + k + 1) * 128],
                                             ftz1, start=False, stop=True)
                        drain(pnm3[:, c:c + 2, :], pt[:])

                # ---- scores -> e2 ----
                e_sb = ezp.tile([128, EW], bf16, tag="e")
                nc.scalar.activation(
                    e_sb[:].rearrange("p (c h) -> p c h", h=4),
                    pnm3[:, :, 64:68], AF.Exp)
                e2 = ezp.tile([128, EW], bf16, tag="e2")
                nc.vector.tensor_tensor(e2[:], e_sb[:],
                                        maske[:, b * EW:(b + 1) * EW],
                                        op=AluOpType.mult)
                # denominators: sum e2 over pixels
                esum = stp.tile([128, 4], f32, tag="esum")
                nc.vector.tensor_reduce(
                    esum[:].unsqueeze(2),
                    e2[:].rearrange("p (c h) -> p h c", h=4), axis=AX,
                    op=AluOpType.add)
                pesT = pssm.tile([4, 128], f32, tag="sm4")
                nc.tensor.transpose(pesT[:], esum[:], i128f[:])
                denom = stp.tile([4, 1], f32, tag="denom")
                nc.vector.tensor_reduce(denom[:].unsqueeze(2),
                                        pesT[:].unsqueeze(1), axis=AX,
                                        op=AluOpType.add)

                # ---- per-pixel stats ----
                mu_raw = stp.tile([128, NCH], f32, tag="mu")
                nc.vector.tensor_copy(mu_raw[:], pnm3[:, :, 68])
                cross = stp.tile([128, NCH], f32, tag="cross")
                nc.vector.tensor_copy(cross[:], pnm3[:, :, 69])
                s2b = stp.tile([128, NCH], f32, tag="s2")
                for g in range(NG):
                    pfg = pnm3[:, g * 8:(g + 1) * 8, 0:64]
                    sq = zp.tile([128, 512], bf16, tag="sq")
                    nc.scalar.activation(
                        sq[:].rearrange("p (c e) -> p c e", e=64), pfg, AF.Square)
                    nc.vector.tensor_reduce(
                        s2b[:, g * 8:(g + 1) * 8].unsqueeze(2),
                        sq[:].rearrange("p (c e) -> p c e", e=64), axis=AX,
                        op=AluOpType.add)
                mu_t = stp.tile([128, NCH], f32, tag="mut")
                nc.vector.tensor_scalar(mu_t[:], mu_raw[:], meanb,
                                        None, op0=AluOpType.add)
                musq = stp.tile([128, NCH], f32, tag="musq")
                nc.vector.tensor_tensor(musq[:], mu_t[:], mu_t[:],
                                        op=AluOpType.mult)
                vb0 = stp.tile([128, NCH], f32, tag="vb0")
                nc.vector.scalar_tensor_tensor(vb0[:], s2b[:], 1.0 / 64.0, musq[:],
                                               op0=AluOpType.mult,
                                               op1=AluOpType.subtract)
                vb = stp.tile([128, NCH], f32, tag="vb")
                nc.vector.scalar_tensor_tensor(vb[:], cross[:], 2.0, vb0[:],
                                               op0=AluOpType.mult,
                                               op1=AluOpType.add)
                stdb = stp.tile([128, NCH], f32, tag="stdb")
                nc.scalar.activation(stdb[:], vb[:], AF.Sqrt, bias=epsm[:], scale=1.0)
                rstd = stp.tile([128, NCH], f32, tag="rstd")
                nc.vector.reciprocal(rstd[:], stdb[:])
                rstdb = stp.tile([128, NCH], bf16, tag="rstdb")
                nc.vector.tensor_copy(rstdb[:], rstd[:])
                return dict(pnm3=pnm3, e2=e2, denom=denom, rstdb=rstdb)

            def emit_tail(b, ct):
                pnm3, e2, denom, rstdb = ct["pnm3"], ct["e2"], ct["denom"], ct["rstdb"]

                # ---- attention context + K tile ----
                pctx = psctxp.tile([4, 64], f32, tag="ctx")
                for c in range(NCH):
                    nc.tensor.matmul(pctx[:], e2[:, c * 4:(c + 1) * 4],
                                     pnm3[:, c, 0:64],
                                     start=(c == 0), stop=(c == NCH - 1))
                ctx_sb = stp.tile([4, 64], f32, tag="ctxs")
                nc.vector.tensor_copy(ctx_sb[:], pctx[:])
                rd = stp.tile([4, 1], f32, tag="rd")
                nc.vector.reciprocal(rd[:], denom[:])
                avg0 = stp.tile([4, 64], f32, tag="avg0")
                nc.vector.tensor_scalar(avg0[:], ctx_sb[:], rd[:], None,
                                        op0=AluOpType.mult)
                avg = stp.tile([4, 64], bf16, tag="avg")
                nc.vector.tensor_tensor(avg[:], avg0[:], cb4[:], op=AluOpType.add)
                pavT = pssm.tile([64, 4], bf16, tag="sm4")
                nc.tensor.transpose(pavT[:], avg[:], i4b[:])
                avT = stp.tile([64, 4], bf16, tag="avT")
                nc.vector.tensor_copy(avT[:], pavT[:])
                psao = pssm.tile([64, 1], f32, tag="sm4")
                for h in range(NH):
                    nc.tensor.matmul(psao[:], mht[:, h * 64:(h + 1) * 64],
                                     avT[:, h:h + 1],
                                     start=(h == 0), stop=(h == NH - 1))
                aocb = stp.tile([64, 1], bf16, tag="aocb")
                nc.scalar.activation(aocb[:], psao[:], AF.Identity,
                                     bias=c0cb[:], scale=1.0)
                pkrow = pssm.tile([1, 64], f32, tag="sm4")
                nc.tensor.matmul(pkrow[:], aocb[:], mt64[:], start=True, stop=True)
                krow = stp.tile([1, 64], bf16, tag="krow")
                nc.vector.tensor_copy(krow[:], pkrow[:])
                pkbc = pssm.tile([128, 64], f32, tag="kbc")
                nc.tensor.matmul(pkbc[:], ones1[:], krow[:], start=True, stop=True)
                kbc = stp.tile([128, 64], bf16, tag="kbcs")
                nc.scalar.activation(kbc[:], pkbc[:], AF.Identity, bias=0.0, scale=1.0)

                # ---- z pipeline (pixel-major groups of 8 chunks) ----
                adj_pm = adjp.tile([128, NCH], f32, tag="adjpm")
                for g in range(NG):
                    zlin = pnm3[:, g * 8:(g + 1) * 8, 70:134]
                    zv = zp.tile([128, 512], bf16, tag="zv")
                    nc.gpsimd.tensor_tensor(
                        zv[:].rearrange("p (c e) -> p c e", e=64), zlin,
                        kbc[:].unsqueeze(1).to_broadcast([128, 8, 64]),
                        op=AluOpType.add)
                    z = zp.tile([128, 512], bf16, tag="z")
                    nc.gpsimd.tensor_tensor(
                        z[:].rearrange("p (c e) -> p c e", e=64),
                        zv[:].rearrange("p (c e) -> p c e", e=64),
                        rstdb[:, g * 8:(g + 1) * 8].unsqueeze(2).to_broadcast([128, 8, 64]),
                        op=AluOpType.mult)
                    z2 = zp.tile([128, 512], bf16, tag="z2")
                    nc.vector.tensor_tensor(z2[:], z[:], b1r[:], op=AluOpType.add)
                    hg = zp.tile([128, 512], bf16, tag="hg")
                    nc.scalar.activation(hg[:], z2[:], AF.Gelu)
                    hw = zp.tile([128, 512], bf16, tag="hw")
                    nc.vector.tensor_tensor(hw[:], hg[:], w2r[:], op=AluOpType.mult)
                    nc.vector.tensor_reduce(
                        adj_pm[:, g * 8:(g + 1) * 8].unsqueeze(2),
                        hw[:].rearrange("p (c e) -> p c e", e=64), axis=AX,
                        op=AluOpType.add)

                # ---- output ----
                padjT = pso.tile([NCH, 128], f32, tag="adjT")
                nc.tensor.transpose(padjT[:], adj_pm[:], i128f[:])
                adj_sb = adjp.tile([NCH, 128], f32, tag="adjsb")
                nc.scalar.activation(adj_sb[:], padjT[:], AF.Identity,
                                     bias=b2c[0:NCH, :], scale=1.0)
                nc.sync.dma_start(OUT[b], adj_sb[:])

            # software pipeline: conv(b+1) is emitted before tail(b) so the
            # PE never waits on batch-b's exp/e2 before starting batch b+1
            convs = [emit_conv(0)]
            for b in range(1, B_PER):
                convs.append(emit_conv(b))
                emit_tail(b - 1, convs[b - 1])
            emit_tail(B_PER - 1, convs[B_PER - 1])

    nc.compile()
    return nc


def _host_prep(inputs):
    """Fold weights exactly as the reference does, in fp32 numpy."""
    import ml_dtypes
    bf = ml_dtypes.bfloat16
    f = lambda x: np.asarray(x, dtype=np.float32)
    conv_w = f(inputs["conv_w"]); conv_b = f(inputs["conv_b"])
    idp_w = f(inputs["idp_w"]); idp_b = f(inputs["idp_b"])
    wq = f(inputs["wq"]); bq = f(inputs["bq"])
    wk = f(inputs["wk"])
    wv = f(inputs["wv"]); bv = f(inputs["bv"])
    wo = f(inputs["wo"]); bo = f(inputs["bo"])
    ln_g = f(inputs["ln_g"]); ln_b = f(inputs["ln_b"])
    w1 = f(inputs["w1"]); b1 = f(inputs["b1"])
    w2 = f(inputs["w2"]); b2 = f(inputs["b2"])
    emb = f(inputs["identity_embs"])
    mask = np.asarray(inputs["contested_mask"]).reshape(N)

    scale = np.float32(1.0 / np.sqrt(np.float32(DH)))
    q = emb @ idp_w.T + idp_b                      # [B, HD]
    qh = (q @ wq.T + bq).reshape(B, NH, DH)
    u = np.einsum('hdk,bhd->bkh', wk.reshape(NH, DH, HD), qh) * scale  # [B,HD,NH]

    W1p = w1 * ln_g[None, :]
    b1p = w1 @ ln_b + b1
    M = W1p - np.outer(W1p @ np.ones(HD, np.float32),
                       np.ones(HD, np.float32)) / HD
    Mh = np.stack([wo[:, h * DH:(h + 1) * DH] @ wv[h * DH:(h + 1) * DH, :]
                   for h in range(NH)])
    c0 = wo @ bv + bo
    A = conv_w                                     # [64, 256]

    # fused weight table per batch: [B, C, FW]
    ftz = np.zeros((B, C, FW), np.float32)
    ftz[:, :, 0:64] = A.T[None]
    ftz[:, :, 64:68] = np.einsum('kc,bkh->bch', A, u)
    ftz[:, :, 68] = (A.T @ (np.ones(HD, np.float32) / HD))[None]
    ftz[:, :, 69] = (A.T @ (conv_b / HD))[None]
    ftz[:, :, 70:134] = (M @ A).T[None]
    ftz_halves = np.stack([ftz[:, 0:128, :], ftz[:, 128:256, :]], axis=1)  # [B,2,128,FW]

    # contested-first pixel permutation (shared mask across batches)
    perm = np.concatenate([np.flatnonzero(mask), np.flatnonzero(~mask)])
    ncon = int(mask.sum())
    nkeep = min(ncon, NCC)
    mask_p = np.zeros(NCC, np.float32)
    mask_p[:nkeep] = 1.0

    # mask multiplier with folded score bias exp(u^T conv_b)
    sbias = np.einsum('k,bkh->bh', conv_b, u)  # [B, NH]
    mf = mask_p.reshape(NCH, 128)  # [c, p], permuted index m = 128c + p
    maskE = np.empty((B, 128, NCH * 4), np.float32)
    for h in range(NH):
        maskE[:, :, h::4] = (mf.T[None] * np.exp(sbias)[:, None, h:h + 1])

    mht = np.concatenate([Mh[h].T for h in range(NH)], axis=1)  # [64, 256]
    mb2 = float((conv_b ** 2).mean())
    consts = dict(
        MHT=mht.astype(bf),
        C0CB=(c0 + conv_b)[:, None].astype(np.float32),
        MT64=M.T.astype(bf),
        CB4=np.repeat(conv_b[None, :], 4, 0).astype(np.float32),
        B1R=np.repeat(np.tile(b1p, 8)[None, :], 128, 0).astype(bf),
        W2R=np.repeat(np.tile(w2[0], 8)[None, :], 128, 0).astype(bf),
        I4=np.eye(4, dtype=np.float32),
        I128=np.eye(128, dtype=np.float32),
        EPSM=np.full((128, 1), 1e-5 + mb2, np.float32),
        B2C=np.full((128, 1), b2[0], np.float32),
    )
    meanb = float(conv_b.mean(dtype=np.float64))
    return ftz_halves.astype(bf), maskE.astype(bf), consts, perm, nkeep, meanb


LAST_RESULTS = None
_MEANB = None


def kernel(**inputs):
    global _BUILT, LAST_RESULTS, _MEANB
    import ml_dtypes
    from concourse.bass_utils import run_bass_kernel_spmd

    ftz_halves, maskE, consts, perm, nkeep, meanb = _host_prep(inputs)

    if _BUILT is None or _MEANB != meanb:
        # meanb is a compile-time scalar folded into an instruction immediate
        _BUILT = _build(meanb)
        _MEANB = meanb
    nc = _BUILT

    pix = np.asarray(inputs["pixel_features"], dtype=np.float32).reshape(B, C, N)
    pix_p = np.zeros((B, C, NCC), np.float32)
    pix_p[:, :, :nkeep] = pix[:, :, perm[:nkeep]]
    pixb = np.stack([pix_p[:, 0:128, :], pix_p[:, 128:256, :]], axis=1).astype(
        ml_dtypes.bfloat16)  # [B, 2, 128, NCC]

    in_maps = []
    for core in range(N_CORES):
        b0 = core * B_PER
        m = dict(consts)
        m["PIX"] = np.ascontiguousarray(pixb[b0:b0 + B_PER])
        m["FTZ"] = np.ascontiguousarray(ftz_halves[b0:b0 + B_PER])
        m["MASKE"] = np.ascontiguousarray(maskE[b0:b0 + B_PER])
        in_maps.append(m)

    res = run_bass_kernel_spmd(nc, in_maps, core_ids=list(range(N_CORES)))
    LAST_RESULTS = res
    adj_p = np.concatenate([res.results[c]["OUT"] for c in range(N_CORES)],
                           axis=0).reshape(B, NCC)
    out = np.zeros((B, N), np.float32)
    out[:, perm[:nkeep]] = adj_p[:, :nkeep]
    return out.reshape(B, H, W)


# revision 36
# speedup vs baseline: 1.0210x; 1.0210x over previous
"""Trainium2 Bass kernel for nn_BoundaryAttention — V3 (fully fused).

Shards batch B=32 across 8 NeuronCores (4 batches per core).

Key idea: ONE pixel-stationary matmul pair per 128-pixel chunk computes, in
pixel-major layout, all linear functions of the raw pixels at once:
  out[pix, 0:64]   = pf_raw      (conv, no bias)
  out[pix, 64:68]  = scores_raw  (folded q/k projections)
  out[pix, 68]     = mu_raw      (feature-mean of pf_raw)
  out[pix, 69]     = cross       (mean(conv_b * pf_raw) term for variance)
  out[pix, 70:134] = zlin_raw    (M-tilde @ pf_raw, LN-mean-fold + W1')
This kills the separate conv, all PE transposes, and the feature-major pfb
buffer of the previous version. Biases are folded downstream:
  - score bias -> exp(bias) folded into the mask multiplier (host)
  - conv_b for attention values -> added to avg post-division (CB4)
  - conv_b/mu/variance shift -> cross column + scalar folds (EPSM)
  - M~(ao + conv_b) -> K broadcast tile added to zlin on device
LayerNorm mean-subtraction is exact via M~ = W1'(I - 11^T/64); variance uses
E[pf^2]-mu^2 (ao cross-terms negligible, validated < 8e-3 rel err).
"""
import numpy as np

B, C, H, W = 32, 256, 128, 128
N = H * W               # 16384
HD, NH, DH = 64, 4, 16
B_PER = 4               # batches per core
N_CORES = 8
NCC = 9216              # contested-pixel capacity (mask ~50% of 16384; +16 sigma pad)
NCH = NCC // 128        # 72 pixel chunks per batch
PIXC = 1536             # pixel DMA chunk columns
FW = 134                # fused output width
NG = NCH // 8           # 10 groups of 8 chunks for the z pipeline

_BUILT = None


def _build(meanb):
    import concourse.bass as bass
    import concourse.mybir as mybir
    import concourse.tile as tile
    import concourse.bacc as bacc
    import bass_rust
    from concourse.alu_op_type import AluOpType

    AF = bass_rust.ActivationFunctionType
    f32 = mybir.dt.float32
    bf16 = mybir.dt.bfloat16
    AX = bass_rust.AxisListType.X

    nc = bacc.Bacc('TRN2', target_bir_lowering=False, debug=False)

    PIX = nc.dram_tensor("PIX", [B_PER, 2, 128, NCC], bf16, kind="ExternalInput")
    FTZ = nc.dram_tensor("FTZ", [B_PER, 2, 128, FW], bf16, kind="ExternalInput")
    MASKE = nc.dram_tensor("MASKE", [B_PER, 128, NCH * 4], bf16, kind="ExternalInput")
    MHT = nc.dram_tensor("MHT", [64, 256], bf16, kind="ExternalInput")
    C0CB = nc.dram_tensor("C0CB", [64, 1], f32, kind="ExternalInput")
    MT64 = nc.dram_tensor("MT64", [64, 64], bf16, kind="ExternalInput")
    CB4 = nc.dram_tensor("CB4", [4, 64], f32, kind="ExternalInput")
    B1R = nc.dram_tensor("B1R", [128, 512], bf16, kind="ExternalInput")
    W2R = nc.dram_tensor("W2R", [128, 512], bf16, kind="ExternalInput")
    I4 = nc.dram_tensor("I4", [4, 4], f32, kind="ExternalInput")
    I128 = nc.dram_tensor("I128", [128, 128], f32, kind="ExternalInput")
    EPSM = nc.dram_tensor("EPSM", [128, 1], f32, kind="ExternalInput")
    B2C = nc.dram_tensor("B2C", [128, 1], f32, kind="ExternalInput")
    OUT = nc.dram_tensor("OUT", [B_PER, NCH, 128], f32, kind="ExternalOutput")

    with tile.TileContext(nc) as tc:
        with tc.tile_pool(name="const", bufs=1) as cpool, \
             tc.tile_pool(name="pix0", bufs=3) as pixp0, \
             tc.tile_pool(name="pix1", bufs=3) as pixp1, \
             tc.tile_pool(name="ez", bufs=2) as ezp, \
             tc.tile_pool(name="zp", bufs=3) as zp, \
             tc.tile_pool(name="st", bufs=2) as stp, \
             tc.tile_pool(name="adj", bufs=2) as adjp, \
             tc.tile_pool(name="ps_f", bufs=3, space="PSUM") as psf, \
             tc.tile_pool(name="ps_ctx", bufs=1, space="PSUM") as psctxp, \
             tc.tile_pool(name="ps_sm", bufs=1, space="PSUM") as pssm, \
             tc.tile_pool(name="ps_o", bufs=1, space="PSUM") as pso:

            # ---- constants ----
            ftz = cpool.tile([128, B_PER * 2 * FW], bf16)
            for _b in range(B_PER):
                for _k in range(2):
                    _o = (_b * 2 + _k) * FW
                    nc.sync.dma_start(ftz[:, _o:_o + FW], FTZ[_b, _k])
            EW = NCH * 4
            maske = cpool.tile([128, B_PER * EW], bf16)
            for _b in range(B_PER):
                nc.sync.dma_start(maske[:, _b * EW:(_b + 1) * EW], MASKE[_b])
            mht = cpool.tile([64, 256], bf16)
            nc.sync.dma_start(mht[:], MHT[:])
            c0cb = cpool.tile([64, 1], f32)
            nc.sync.dma_start(c0cb[:], C0CB[:])
            mt64 = cpool.tile([64, 64], bf16)
            nc.sync.dma_start(mt64[:], MT64[:])
            cb4 = cpool.tile([4, 64], f32)
            nc.sync.dma_start(cb4[:], CB4[:])
            b1r = cpool.tile([128, 512], bf16)
            nc.sync.dma_start(b1r[:], B1R[:])
            w2r = cpool.tile([128, 512], bf16)
            nc.sync.dma_start(w2r[:], W2R[:])
            i4f = cpool.tile([4, 4], f32)
            nc.sync.dma_start(i4f[:], I4[:])
            i4b = cpool.tile([4, 4], bf16)
            nc.vector.tensor_copy(i4b[:], i4f[:])
            i128f = cpool.tile([128, 128], f32)
            nc.sync.dma_start(i128f[:], I128[:])
            epsm = cpool.tile([128, 1], f32)
            nc.sync.dma_start(epsm[:], EPSM[:])
            b2c = cpool.tile([128, 1], f32)
            nc.sync.dma_start(b2c[:], B2C[:])
            ones1 = cpool.tile([1, 128], bf16)
            nc.vector.memset(ones1[:], 1.0)

            # persistent pixel-major fused outputs, double-buffered over batches
            pnm_bufs = [cpool.tile([128, NCH * FW], bf16, name=f"pnm{i}")
                        for i in range(2)]

            drain_idx = [0]

            def drain(dst, src):
                # psum->sbuf drains: 1/3 DVE, 2/3 Act (GPSIMD cannot touch PSUM)
                k = drain_idx[0]
                drain_idx[0] += 1
                if k % 3 == 0:
                    nc.vector.tensor_copy(dst, src)
                else:
                    nc.scalar.activation(dst, src, AF.Copy, bias=0.0)

            def emit_batch(b):
                pnm = pnm_bufs[b % 2]
                pnm3 = pnm[:].rearrange("p (c e) -> p c e", e=FW)
                ftz0 = ftz[:, (b * 2) * FW:(b * 2 + 1) * FW]
                ftz1 = ftz[:, (b * 2 + 1) * FW:(b * 2 + 2) * FW]

                # ---- fused conv/scores/stats/zlin, pixel-stationary ----
                for pos in range(NCC // PIXC):
                    px0 = pixp0.tile([128, PIXC], bf16, tag="px0")
                    px1 = pixp1.tile([128, PIXC], bf16, tag="px1")
                    nc.sync.dma_start(px0[:], PIX[b, 0, :, pos * PIXC:(pos + 1) * PIXC])
                    nc.sync.dma_start(px1[:], PIX[b, 1, :, pos * PIXC:(pos + 1) * PIXC])
                    for s in range(0, PIXC // 128, 2):
                        c = pos * (PIXC // 128) + s
                        pt = psf.tile([128, 2, FW], f32, tag="pfz")
                        for k in range(2):
                            nc.tensor.matmul(pt[:, k, :],
                                             px0[:, (s + k) * 128:(s + k + 1) * 128],
                                             ftz0, start=True, stop=False)
                            nc.tensor.matmul(pt[:, k, :],
                                             px1[:, (s + k) * 128:(s + k + 1) * 128],
                                             ftz1, start=False, stop=True)
                        drain(pnm3[:, c:c + 2, :], pt[:])

                # ---- scores -> e2 ----
                e_sb = ezp.tile([128, EW], bf16, tag="e")
                nc.scalar.activation(
                    e_sb[:].rearrange("p (c h) -> p c h", h=4),
                    pnm3[:, :, 64:68], AF.Exp)
                e2 = ezp.tile([128, EW], bf16, tag="e2")
                nc.vector.tensor_tensor(e2[:], e_sb[:],
                                        maske[:, b * EW:(b + 1) * EW],
                                        op=AluOpType.mult)
                # denominators: sum e2 over pixels
                esum = stp.tile([128, 4], f32, tag="esum")
                nc.vector.tensor_reduce(
                    esum[:].unsqueeze(2),
                    e2[:].rearrange("p (c h) -> p h c", h=4), axis=AX,
                    op=AluOpType.add)
                pesT = pssm.tile([4, 128], f32, tag="sm4")
                nc.tensor.transpose(pesT[:], esum[:], i128f[:])
                denom = stp.tile([4, 1], f32, tag="denom")
                nc.vector.tensor_reduce(denom[:].unsqueeze(2),
                                        pesT[:].unsqueeze(1), axis=AX,
                                        op=AluOpType.add)

                # ---- per-pixel stats ----
                mu_raw = stp.tile([128, NCH], f32, tag="mu")
                nc.vector.tensor_copy(mu_raw[:], pnm3[:, :, 68])
                cross = stp.tile([128, NCH], f32, tag="cross")
                nc.vector.tensor_copy(cross[:], pnm3[:, :, 69])
                s2b = stp.tile([128, NCH], f32, tag="s2")
                for g in range(NG):
                    pfg = pnm3[:, g * 8:(g + 1) * 8, 0:64]
                    sq = zp.tile([128, 512], bf16, tag="sq")
                    nc.scalar.activation(
                        sq[:].rearrange("p (c e) -> p c e", e=64), pfg, AF.Square)
                    nc.vector.tensor_reduce(
                        s2b[:, g * 8:(g + 1) * 8].unsqueeze(2),
                        sq[:].rearrange("p (c e) -> p c e", e=64), axis=AX,
                        op=AluOpType.add)
                mu_t = stp.tile([128, NCH], f32, tag="mut")
                nc.vector.tensor_scalar(mu_t[:], mu_raw[:], meanb,
                                        None, op0=AluOpType.add)
                musq = stp.tile([128, NCH], f32, tag="musq")
                nc.vector.tensor_tensor(musq[:], mu_t[:], mu_t[:],
                                        op=AluOpType.mult)
                vb0 = stp.tile([128, NCH], f32, tag="vb0")
                nc.vector.scalar_tensor_tensor(vb0[:], s2b[:], 1.0 / 64.0, musq[:],
                                               op0=AluOpType.mult,
                                               op1=AluOpType.subtract)
                vb = stp.tile([128, NCH], f32, tag="vb")
                nc.vector.scalar_tensor_tensor(vb[:], cross[:], 2.0, vb0[:],
                                               op0=AluOpType.mult,
                                               op1=AluOpType.add)
                stdb = stp.tile([128, NCH], f32, tag="stdb")
                nc.scalar.activation(stdb[:], vb[:], AF.Sqrt, bias=epsm[:], scale=1.0)
                rstd = stp.tile([128, NCH], f32, tag="rstd")
                nc.vector.reciprocal(rstd[:], stdb[:])
                rstdb = stp.tile([128, NCH], bf16, tag="rstdb")
                nc.vector.tensor_copy(rstdb[:], rstd[:])

                # ---- attention context + K tile ----
                pctx = psctxp.tile([4, 64], f32, tag="ctx")
                for c in range(NCH):
                    nc.tensor.matmul(pctx[:], e2[:, c * 4:(c + 1) * 4],
                                     pnm3[:, c, 0:64],
                                     start=(c == 0), stop=(c == NCH - 1))
                ctx_sb = stp.tile([4, 64], f32, tag="ctxs")
                nc.vector.tensor_copy(ctx_sb[:], pctx[:])
                rd = stp.tile([4, 1], f32, tag="rd")
                nc.vector.reciprocal(rd[:], denom[:])
                avg0 = stp.tile([4, 64], f32, tag="avg0")
                nc.vector.tensor_scalar(avg0[:], ctx_sb[:], rd[:], None,
                                        op0=AluOpType.mult)
                avg = stp.tile([4, 64], bf16, tag="avg")
                nc.vector.tensor_tensor(avg[:], avg0[:], cb4[:], op=AluOpType.add)
                pavT = pssm.tile([64, 4], bf16, tag="sm4")
                nc.tensor.transpose(pavT[:], avg[:], i4b[:])
                avT = stp.tile([64, 4], bf16, tag="avT")
                nc.vector.tensor_copy(avT[:], pavT[:])
                psao = pssm.tile([64, 1], f32, tag="sm4")
                for h in range(NH):
                    nc.tensor.matmul(psao[:], mht[:, h * 64:(h + 1) * 64],
                                     avT[:, h:h + 1],
                                     start=(h == 0), stop=(h == NH - 1))
                aocb = stp.tile([64, 1], bf16, tag="aocb")
                nc.scalar.activation(aocb[:], psao[:], AF.Identity,
                                     bias=c0cb[:], scale=1.0)
                pkrow = pssm.tile([1, 64], f32, tag="sm4")
                nc.tensor.matmul(pkrow[:], aocb[:], mt64[:], start=True, stop=True)
                krow = stp.tile([1, 64], bf16, tag="krow")
                nc.vector.tensor_copy(krow[:], pkrow[:])
                pkbc = pssm.tile([128, 64], f32, tag="kbc")
                nc.tensor.matmul(pkbc[:], ones1[:], krow[:], start=True, stop=True)
                kbc = stp.tile([128, 64], bf16, tag="kbcs")
                nc.scalar.activation(kbc[:], pkbc[:], AF.Identity, bias=0.0, scale=1.0)

                # ---- z pipeline (pixel-major groups of 8 chunks) ----
                adj_pm = adjp.tile([128, NCH], f32, tag="adjpm")
                for g in range(NG):
                    zlin = pnm3[:, g * 8:(g + 1) * 8, 70:134]
                    zv = zp.tile([128, 512], bf16, tag="zv")
                    nc.gpsimd.tensor_tensor(
                        zv[:].rearrange("p (c e) -> p c e", e=64), zlin,
                        kbc[:].unsqueeze(1).to_broadcast([128, 8, 64]),
                        op=AluOpType.add)
                    z = zp.tile([128, 512], bf16, tag="z")
                    nc.gpsimd.tensor_tensor(
                        z[:].rearrange("p (c e) -> p c e", e=64),
                        zv[:].rearrange("p (c e) -> p c e", e=64),
                        rstdb[:, g * 8:(g + 1) * 8].unsqueeze(2).to_broadcast([128, 8, 64]),
                        op=AluOpType.mult)
                    z2 = zp.tile([128, 512], bf16, tag="z2")
                    nc.vector.tensor_tensor(z2[:], z[:], b1r[:], op=AluOpType.add)
                    hg = zp.tile([128, 512], bf16, tag="hg")
                    nc.scalar.activation(hg[:], z2[:], AF.Gelu)
                    hw = zp.tile([128, 512], bf16, tag="hw")
                    nc.vector.tensor_tensor(hw[:], hg[:], w2r[:], op=AluOpType.mult)
                    nc.vector.tensor_reduce(
                        adj_pm[:, g * 8:(g + 1) * 8].unsqueeze(2),
                        hw[:].rearrange("p (c e) -> p c e", e=64), axis=AX,
                        op=AluOpType.add)

                # ---- output ----
                padjT = pso.tile([NCH, 128], f32, tag="adjT")
                nc.tensor.transpose(padjT[:], adj_pm[:], i128f[:])
                adj_sb = adjp.tile([NCH, 128], f32, tag="adjsb")
                nc.scalar.activation(adj_sb[:], padjT[:], AF.Identity,
                                     bias=b2c[0:NCH, :], scale=1.0)
                nc.sync.dma_start(OUT[b], adj_sb[:])

            for b in range(B_PER):
                emit_batch(b)

    nc.compile()
    return nc


def _host_prep(inputs):
    """Fold weights exactly as the reference does, in fp32 numpy."""
    import ml_dtypes
    bf = ml_dtypes.bfloat16
    f = lambda x: np.asarray(x, dtype=np.float32)
    conv_w = f(inputs["conv_w"]); conv_b = f(inputs["conv_b"])
    idp_w = f(inputs["idp_w"]); idp_b = f(inputs["idp_b"])
    wq = f(inputs["wq"]); bq = f(inputs["bq"])
    wk = f(inputs["wk"])
    wv = f(inputs["wv"]); bv = f(inputs["bv"])
    wo = f(inputs["wo"]); bo = f(inputs["bo"])
    ln_g = f(inputs["ln_g"]); ln_b = f(inputs["ln_b"])
    w1 = f(inputs["w1"]); b1 = f(inputs["b1"])
    w2 = f(inputs["w2"]); b2 = f(inputs["b2"])
    emb = f(inputs["identity_embs"])
    mask = np.asarray(inputs["contested_mask"]).reshape(N)

    scale = np.float32(1.0 / np.sqrt(np.float32(DH)))
    q = emb @ idp_w.T + idp_b                      # [B, HD]
    qh = (q @ wq.T + bq).reshape(B, NH, DH)
    u = np.einsum('hdk,bhd->bkh', wk.reshape(NH, DH, HD), qh) * scale  # [B,HD,NH]

    W1p = w1 * ln_g[None, :]
    b1p = w1 @ ln_b + b1
    M = W1p - np.outer(W1p @ np.ones(HD, np.float32),
                       np.ones(HD, np.float32)) / HD
    Mh = np.stack([wo[:, h * DH:(h + 1) * DH] @ wv[h * DH:(h + 1) * DH, :]
                   for h in range(NH)])
    c0 = wo @ bv + bo
    A = conv_w                                     # [64, 256]

    # fused weight table per batch: [B, C, FW]
    ftz = np.zeros((B, C, FW), np.float32)
    ftz[:, :, 0:64] = A.T[None]
    ftz[:, :, 64:68] = np.einsum('kc,bkh->bch', A, u)
    ftz[:, :, 68] = (A.T @ (np.ones(HD, np.float32) / HD))[None]
    ftz[:, :, 69] = (A.T @ (conv_b / HD))[None]
    ftz[:, :, 70:134] = (M @ A).T[None]
    ftz_halves = np.stack([ftz[:, 0:128, :], ftz[:, 128:256, :]], axis=1)  # [B,2,128,FW]

    # contested-first pixel permutation (shared mask across batches)
    perm = np.concatenate([np.flatnonzero(mask), np.flatnonzero(~mask)])
    ncon = int(mask.sum())
    nkeep = min(ncon, NCC)
    mask_p = np.zeros(NCC, np.float32)
    mask_p[:nkeep] = 1.0

    # mask multiplier with folded score bias exp(u^T conv_b)
    sbias = np.einsum('k,bkh->bh', conv_b, u)  # [B, NH]
    mf = mask_p.reshape(NCH, 128)  # [c, p], permuted index m = 128c + p
    maskE = np.empty((B, 128, NCH * 4), np.float32)
    for h in range(NH):
        maskE[:, :, h::4] = (mf.T[None] * np.exp(sbias)[:, None, h:h + 1])

    mht = np.concatenate([Mh[h].T for h in range(NH)], axis=1)  # [64, 256]
    mb2 = float((conv_b ** 2).mean())
    consts = dict(
        MHT=mht.astype(bf),
        C0CB=(c0 + conv_b)[:, None].astype(np.float32),
        MT64=M.T.astype(bf),
        CB4=np.repeat(conv_b[None, :], 4, 0).astype(np.float32),
        B1R=np.repeat(np.tile(b1p, 8)[None, :], 128, 0).astype(bf),
        W2R=np.repeat(np.tile(w2[0], 8)[None, :], 128, 0).astype(bf),
        I4=np.eye(4, dtype=np.float32),
        I128=np.eye(128, dtype=np.float32),
        EPSM=np.full((128, 1), 1e-5 + mb2, np.float32),
        B2C=np.full((128, 1), b2[0], np.float32),
    )
    meanb = float(conv_b.mean(dtype=np.float64))
    return ftz_halves.astype(bf), maskE.astype(bf), consts, perm, nkeep, meanb


LAST_RESULTS = None
_MEANB = None


def kernel(**inputs):
    global _BUILT, LAST_RESULTS, _MEANB
    import ml_dtypes
    from concourse.bass_utils import run_bass_kernel_spmd

    ftz_halves, maskE, consts, perm, nkeep, meanb = _host_prep(inputs)

    if _BUILT is None or _MEANB != meanb:
        # meanb is a compile-time scalar folded into an instruction immediate
        _BUILT = _build(meanb)
        _MEANB = meanb
    nc = _BUILT

    pix = np.asarray(inputs["pixel_features"], dtype=np.float32).reshape(B, C, N)
    pix_p = np.zeros((B, C, NCC), np.float32)
    pix_p[:, :, :nkeep] = pix[:, :, perm[:nkeep]]
    pixb = np.stack([pix_p[:, 0:128, :], pix_p[:, 128:256, :]], axis=1).astype(
        ml_dtypes.bfloat16)  # [B, 2, 128, NCC]

    in_maps = []
    for core in range(N_CORES):
        b0 = core * B_PER
        m = dict(consts)
        m["PIX"] = np.ascontiguousarray(pixb[b0:b0 + B_PER])
        m["FTZ"] = np.ascontiguousarray(ftz_halves[b0:b0 + B_PER])
        m["MASKE"] = np.ascontiguousarray(maskE[b0:b0 + B_PER])
        in_maps.append(m)

    res = run_bass_kernel_spmd(nc, in_maps, core_ids=list(range(N_CORES)))
    LAST_RESULTS = res
    adj_p = np.concatenate([res.results[c]["OUT"] for c in range(N_CORES)],
                           axis=0).reshape(B, NCC)
    out = np.zeros((B, N), np.float32)
    out[:, perm[:nkeep]] = adj_p[:, :nkeep]
    return out.reshape(B, H, W)


# revision 38
# speedup vs baseline: 1.0598x; 1.0381x over previous
"""Trainium2 Bass kernel for nn_BoundaryAttention — V3 (fully fused).

Shards batch B=32 across 8 NeuronCores (4 batches per core).

Key idea: ONE pixel-stationary matmul pair per 128-pixel chunk computes, in
pixel-major layout, all linear functions of the raw pixels at once:
  out[pix, 0:64]   = pf_raw      (conv, no bias)
  out[pix, 64:68]  = scores_raw  (folded q/k projections)
  out[pix, 68]     = mu_raw      (feature-mean of pf_raw)
  out[pix, 69]     = cross       (mean(conv_b * pf_raw) term for variance)
  out[pix, 70:134] = zlin_raw    (M-tilde @ pf_raw, LN-mean-fold + W1')
This kills the separate conv, all PE transposes, and the feature-major pfb
buffer of the previous version. Biases are folded downstream:
  - score bias -> exp(bias) folded into the mask multiplier (host)
  - conv_b for attention values -> added to avg post-division (CB4)
  - conv_b/mu/variance shift -> cross column + scalar folds (EPSM)
  - M~(ao + conv_b) -> K broadcast tile added to zlin on device
LayerNorm mean-subtraction is exact via M~ = W1'(I - 11^T/64); variance uses
E[pf^2]-mu^2 (ao cross-terms negligible, validated < 8e-3 rel err).
"""
import numpy as np

B, C, H, W = 32, 256, 128, 128
N = H * W               # 16384
HD, NH, DH = 64, 4, 16
B_PER = 4               # batches per core
N_CORES = 8
NCC = 9216              # contested-pixel capacity (mask ~50% of 16384; +16 sigma pad)
NCH = NCC // 128        # 72 pixel chunks per batch
PIXC = 1536             # pixel DMA chunk columns
FW = 134                # fused output width
NG = NCH // 8           # 10 groups of 8 chunks for the z pipeline

_BUILT = None


def _build(meanb):
    import concourse.bass as bass
    import concourse.mybir as mybir
    import concourse.tile as tile
    import concourse.bacc as bacc
    import bass_rust
    from concourse.alu_op_type import AluOpType

    AF = bass_rust.ActivationFunctionType
    f32 = mybir.dt.float32
    bf16 = mybir.dt.bfloat16
    AX = bass_rust.AxisListType.X

    nc = bacc.Bacc('TRN2', target_bir_lowering=False, debug=False)

    PIX = nc.dram_tensor("PIX", [B_PER, 2, 128, NCC], bf16, kind="ExternalInput")
    FTZ = nc.dram_tensor("FTZ", [B_PER, 2, 128, FW], bf16, kind="ExternalInput")
    MASKE = nc.dram_tensor("MASKE", [B_PER, 128, NCH * 4], bf16, kind="ExternalInput")
    MHT = nc.dram_tensor("MHT", [64, 256], bf16, kind="ExternalInput")
    C0CB = nc.dram_tensor("C0CB", [64, 1], f32, kind="ExternalInput")
    MT64 = nc.dram_tensor("MT64", [64, 64], bf16, kind="ExternalInput")
    CB4 = nc.dram_tensor("CB4", [4, 64], f32, kind="ExternalInput")
    B1R = nc.dram_tensor("B1R", [128, 512], bf16, kind="ExternalInput")
    W2R = nc.dram_tensor("W2R", [128, 512], bf16, kind="ExternalInput")
    I4 = nc.dram_tensor("I4", [4, 4], f32, kind="ExternalInput")
    I128 = nc.dram_tensor("I128", [128, 128], f32, kind="ExternalInput")
    EPSM = nc.dram_tensor("EPSM", [128, 1], f32, kind="ExternalInput")
    B2C = nc.dram_tensor("B2C", [128, 1], f32, kind="ExternalInput")
    OUT = nc.dram_tensor("OUT", [B_PER, NCH, 128], f32, kind="ExternalOutput")

    with tile.TileContext(nc) as tc:
        with tc.tile_pool(name="const", bufs=1) as cpool, \
             tc.tile_pool(name="pix0", bufs=3) as pixp0, \
             tc.tile_pool(name="pix1", bufs=3) as pixp1, \
             tc.tile_pool(name="ez", bufs=2) as ezp, \
             tc.tile_pool(name="zp", bufs=3) as zp, \
             tc.tile_pool(name="st", bufs=2) as stp, \
             tc.tile_pool(name="adj", bufs=2) as adjp, \
             tc.tile_pool(name="ps_f", bufs=3, space="PSUM") as psf, \
             tc.tile_pool(name="ps_ctx", bufs=1, space="PSUM") as psctxp, \
             tc.tile_pool(name="ps_sm", bufs=1, space="PSUM") as pssm, \
             tc.tile_pool(name="ps_o", bufs=1, space="PSUM") as pso:

            # ---- constants ----
            ftz = cpool.tile([128, B_PER * 2 * FW], bf16)
            for _b in range(B_PER):
                for _k in range(2):
                    _o = (_b * 2 + _k) * FW
                    nc.sync.dma_start(ftz[:, _o:_o + FW], FTZ[_b, _k])
            EW = NCH * 4
            maske = cpool.tile([128, B_PER * EW], bf16)
            for _b in range(B_PER):
                nc.sync.dma_start(maske[:, _b * EW:(_b + 1) * EW], MASKE[_b])
            mht = cpool.tile([64, 256], bf16)
            nc.sync.dma_start(mht[:], MHT[:])
            c0cb = cpool.tile([64, 1], f32)
            nc.sync.dma_start(c0cb[:], C0CB[:])
            mt64 = cpool.tile([64, 64], bf16)
            nc.sync.dma_start(mt64[:], MT64[:])
            cb4 = cpool.tile([4, 64], f32)
            nc.sync.dma_start(cb4[:], CB4[:])
            b1r = cpool.tile([128, 512], bf16)
            nc.sync.dma_start(b1r[:], B1R[:])
            w2r = cpool.tile([128, 512], bf16)
            nc.sync.dma_start(w2r[:], W2R[:])
            i4f = cpool.tile([4, 4], f32)
            nc.sync.dma_start(i4f[:], I4[:])
            i4b = cpool.tile([4, 4], bf16)
            nc.vector.tensor_copy(i4b[:], i4f[:])
            i128f = cpool.tile([128, 128], f32)
            nc.sync.dma_start(i128f[:], I128[:])
            epsm = cpool.tile([128, 1], f32)
            nc.sync.dma_start(epsm[:], EPSM[:])
            b2c = cpool.tile([128, 1], f32)
            nc.sync.dma_start(b2c[:], B2C[:])
            ones1 = cpool.tile([1, 128], bf16)
            nc.vector.memset(ones1[:], 1.0)

            # persistent pixel-major fused outputs, double-buffered over batches
            pnm_bufs = [cpool.tile([128, NCH * FW], bf16, name=f"pnm{i}")
                        for i in range(2)]

            drain_idx = [0]

            def drain(dst, src):
                # psum->sbuf drains: ~1/7 DVE, rest Act (GPSIMD cannot touch
                # PSUM); weighting balances measured DVE/Act busy times
                k = drain_idx[0]
                drain_idx[0] += 1
                if k % 7 == 0:
                    nc.vector.tensor_copy(dst, src)
                else:
                    nc.scalar.activation(dst, src, AF.Copy, bias=0.0)

            def emit_batch(b):
                pnm = pnm_bufs[b % 2]
                pnm3 = pnm[:].rearrange("p (c e) -> p c e", e=FW)
                ftz0 = ftz[:, (b * 2) * FW:(b * 2 + 1) * FW]
                ftz1 = ftz[:, (b * 2 + 1) * FW:(b * 2 + 2) * FW]

                # ---- fused conv/scores/stats/zlin, pixel-stationary ----
                for pos in range(NCC // PIXC):
                    px0 = pixp0.tile([128, PIXC], bf16, tag="px0")
                    px1 = pixp1.tile([128, PIXC], bf16, tag="px1")
                    nc.sync.dma_start(px0[:], PIX[b, 0, :, pos * PIXC:(pos + 1) * PIXC])
                    nc.sync.dma_start(px1[:], PIX[b, 1, :, pos * PIXC:(pos + 1) * PIXC])
                    for s in range(0, PIXC // 128, 3):
                        c = pos * (PIXC // 128) + s
                        pt = psf.tile([128, 3, FW], f32, tag="pfz")
                        for k in range(3):
                            nc.tensor.matmul(pt[:, k, :],
                                             px0[:, (s + k) * 128:(s + k + 1) * 128],
                                             ftz0, start=True, stop=False)
                            nc.tensor.matmul(pt[:, k, :],
                                             px1[:, (s + k) * 128:(s + k + 1) * 128],
                                             ftz1, start=False, stop=True)
                        drain(pnm3[:, c:c + 3, :], pt[:])

                # ---- scores -> e2 ----
                e_sb = ezp.tile([128, EW], bf16, tag="e")
                nc.scalar.activation(
                    e_sb[:].rearrange("p (c h) -> p c h", h=4),
                    pnm3[:, :, 64:68], AF.Exp)
                e2 = ezp.tile([128, EW], bf16, tag="e2")
                nc.vector.tensor_tensor(e2[:], e_sb[:],
                                        maske[:, b * EW:(b + 1) * EW],
                                        op=AluOpType.mult)
                # denominators: sum e2 over pixels
                esum = stp.tile([128, 4], f32, tag="esum")
                nc.vector.tensor_reduce(
                    esum[:].unsqueeze(2),
                    e2[:].rearrange("p (c h) -> p h c", h=4), axis=AX,
                    op=AluOpType.add)
                pesT = pssm.tile([4, 128], f32, tag="sm4")
                nc.tensor.transpose(pesT[:], esum[:], i128f[:])
                denom = stp.tile([4, 1], f32, tag="denom")
                nc.vector.tensor_reduce(denom[:].unsqueeze(2),
                                        pesT[:].unsqueeze(1), axis=AX,
                                        op=AluOpType.add)

                # ---- per-pixel stats ----
                mu_raw = stp.tile([128, NCH], f32, tag="mu")
                nc.vector.tensor_copy(mu_raw[:], pnm3[:, :, 68])
                cross = stp.tile([128, NCH], f32, tag="cross")
                nc.vector.tensor_copy(cross[:], pnm3[:, :, 69])
                s2b = stp.tile([128, NCH], f32, tag="s2")
                for g in range(NG):
                    pfg = pnm3[:, g * 8:(g + 1) * 8, 0:64]
                    sq = zp.tile([128, 512], bf16, tag="sq")
                    nc.scalar.activation(
                        sq[:].rearrange("p (c e) -> p c e", e=64), pfg, AF.Square)
                    nc.vector.tensor_reduce(
                        s2b[:, g * 8:(g + 1) * 8].unsqueeze(2),
                        sq[:].rearrange("p (c e) -> p c e", e=64), axis=AX,
                        op=AluOpType.add)
                mu_t = stp.tile([128, NCH], f32, tag="mut")
                nc.vector.tensor_scalar(mu_t[:], mu_raw[:], meanb,
                                        None, op0=AluOpType.add)
                musq = stp.tile([128, NCH], f32, tag="musq")
                nc.vector.tensor_tensor(musq[:], mu_t[:], mu_t[:],
                                        op=AluOpType.mult)
                vb0 = stp.tile([128, NCH], f32, tag="vb0")
                nc.vector.scalar_tensor_tensor(vb0[:], s2b[:], 1.0 / 64.0, musq[:],
                                               op0=AluOpType.mult,
                                               op1=AluOpType.subtract)
                vb = stp.tile([128, NCH], f32, tag="vb")
                nc.vector.scalar_tensor_tensor(vb[:], cross[:], 2.0, vb0[:],
                                               op0=AluOpType.mult,
                                               op1=AluOpType.add)
                stdb = stp.tile([128, NCH], f32, tag="stdb")
                nc.scalar.activation(stdb[:], vb[:], AF.Sqrt, bias=epsm[:], scale=1.0)
                rstd = stp.tile([128, NCH], f32, tag="rstd")
                nc.vector.reciprocal(rstd[:], stdb[:])
                rstdb = stp.tile([128, NCH], bf16, tag="rstdb")
                nc.vector.tensor_copy(rstdb[:], rstd[:])

                # ---- attention context + K tile ----
                pctx = psctxp.tile([4, 64], f32, tag="ctx")
                for c in range(NCH):
                    nc.tensor.matmul(pctx[:], e2[:, c * 4:(c + 1) * 4],
                                     pnm3[:, c, 0:64],
                                     start=(c == 0), stop=(c == NCH - 1))
                ctx_sb = stp.tile([4, 64], f32, tag="ctxs")
                nc.vector.tensor_copy(ctx_sb[:], pctx[:])
                rd = stp.tile([4, 1], f32, tag="rd")
                nc.vector.reciprocal(rd[:], denom[:])
                avg0 = stp.tile([4, 64], f32, tag="avg0")
                nc.vector.tensor_scalar(avg0[:], ctx_sb[:], rd[:], None,
                                        op0=AluOpType.mult)
                avg = stp.tile([4, 64], bf16, tag="avg")
                nc.vector.tensor_tensor(avg[:], avg0[:], cb4[:], op=AluOpType.add)
                pavT = pssm.tile([64, 4], bf16, tag="sm4")
                nc.tensor.transpose(pavT[:], avg[:], i4b[:])
                avT = stp.tile([64, 4], bf16, tag="avT")
                nc.vector.tensor_copy(avT[:], pavT[:])
                psao = pssm.tile([64, 1], f32, tag="sm4")
                for h in range(NH):
                    nc.tensor.matmul(psao[:], mht[:, h * 64:(h + 1) * 64],
                                     avT[:, h:h + 1],
                                     start=(h == 0), stop=(h == NH - 1))
                aocb = stp.tile([64, 1], bf16, tag="aocb")
                nc.scalar.activation(aocb[:], psao[:], AF.Identity,
                                     bias=c0cb[:], scale=1.0)
                pkrow = pssm.tile([1, 64], f32, tag="sm4")
                nc.tensor.matmul(pkrow[:], aocb[:], mt64[:], start=True, stop=True)
                krow = stp.tile([1, 64], bf16, tag="krow")
                nc.vector.tensor_copy(krow[:], pkrow[:])
                pkbc = pssm.tile([128, 64], f32, tag="kbc")
                nc.tensor.matmul(pkbc[:], ones1[:], krow[:], start=True, stop=True)
                kbc = stp.tile([128, 64], bf16, tag="kbcs")
                nc.scalar.activation(kbc[:], pkbc[:], AF.Identity, bias=0.0, scale=1.0)

                # ---- z pipeline (pixel-major groups of 8 chunks) ----
                adj_pm = adjp.tile([128, NCH], f32, tag="adjpm")
                for g in range(NG):
                    zlin = pnm3[:, g * 8:(g + 1) * 8, 70:134]
                    zv = zp.tile([128, 512], bf16, tag="zv")
                    nc.gpsimd.tensor_tensor(
                        zv[:].rearrange("p (c e) -> p c e", e=64), zlin,
                        kbc[:].unsqueeze(1).to_broadcast([128, 8, 64]),
                        op=AluOpType.add)
                    z = zp.tile([128, 512], bf16, tag="z")
                    nc.gpsimd.tensor_tensor(
                        z[:].rearrange("p (c e) -> p c e", e=64),
                        zv[:].rearrange("p (c e) -> p c e", e=64),
                        rstdb[:, g * 8:(g + 1) * 8].unsqueeze(2).to_broadcast([128, 8, 64]),
                        op=AluOpType.mult)
                    z2 = zp.tile([128, 512], bf16, tag="z2")
                    nc.vector.tensor_tensor(z2[:], z[:], b1r[:], op=AluOpType.add)
                    hg = zp.tile([128, 512], bf16, tag="hg")
                    nc.scalar.activation(hg[:], z2[:], AF.Gelu)
                    hw = zp.tile([128, 512], bf16, tag="hw")
                    nc.vector.tensor_tensor(hw[:], hg[:], w2r[:], op=AluOpType.mult)
                    nc.vector.tensor_reduce(
                        adj_pm[:, g * 8:(g + 1) * 8].unsqueeze(2),
                        hw[:].rearrange("p (c e) -> p c e", e=64), axis=AX,
                        op=AluOpType.add)

                # ---- output ----
                padjT = pso.tile([NCH, 128], f32, tag="adjT")
                nc.tensor.transpose(padjT[:], adj_pm[:], i128f[:])
                adj_sb = adjp.tile([NCH, 128], f32, tag="adjsb")
                nc.scalar.activation(adj_sb[:], padjT[:], AF.Identity,
                                     bias=b2c[0:NCH, :], scale=1.0)
                nc.sync.dma_start(OUT[b], adj_sb[:])

            for b in range(B_PER):
                emit_batch(b)

    nc.compile()
    return nc


def _host_prep(inputs):
    """Fold weights exactly as the reference does, in fp32 numpy."""
    import ml_dtypes
    bf = ml_dtypes.bfloat16
    f = lambda x: np.asarray(x, dtype=np.float32)
    conv_w = f(inputs["conv_w"]); conv_b = f(inputs["conv_b"])
    idp_w = f(inputs["idp_w"]); idp_b = f(inputs["idp_b"])
    wq = f(inputs["wq"]); bq = f(inputs["bq"])
    wk = f(inputs["wk"])
    wv = f(inputs["wv"]); bv = f(inputs["bv"])
    wo = f(inputs["wo"]); bo = f(inputs["bo"])
    ln_g = f(inputs["ln_g"]); ln_b = f(inputs["ln_b"])
    w1 = f(inputs["w1"]); b1 = f(inputs["b1"])
    w2 = f(inputs["w2"]); b2 = f(inputs["b2"])
    emb = f(inputs["identity_embs"])
    mask = np.asarray(inputs["contested_mask"]).reshape(N)

    scale = np.float32(1.0 / np.sqrt(np.float32(DH)))
    q = emb @ idp_w.T + idp_b                      # [B, HD]
    qh = (q @ wq.T + bq).reshape(B, NH, DH)
    u = np.einsum('hdk,bhd->bkh', wk.reshape(NH, DH, HD), qh) * scale  # [B,HD,NH]

    W1p = w1 * ln_g[None, :]
    b1p = w1 @ ln_b + b1
    M = W1p - np.outer(W1p @ np.ones(HD, np.float32),
                       np.ones(HD, np.float32)) / HD
    Mh = np.stack([wo[:, h * DH:(h + 1) * DH] @ wv[h * DH:(h + 1) * DH, :]
                   for h in range(NH)])
    c0 = wo @ bv + bo
    A = conv_w                                     # [64, 256]

    # fused weight table per batch: [B, C, FW]
    ftz = np.zeros((B, C, FW), np.float32)
    ftz[:, :, 0:64] = A.T[None]
    ftz[:, :, 64:68] = np.einsum('kc,bkh->bch', A, u)
    ftz[:, :, 68] = (A.T @ (np.ones(HD, np.float32) / HD))[None]
    ftz[:, :, 69] = (A.T @ (conv_b / HD))[None]
    ftz[:, :, 70:134] = (M @ A).T[None]
    ftz_halves = np.stack([ftz[:, 0:128, :], ftz[:, 128:256, :]], axis=1)  # [B,2,128,FW]

    # contested-first pixel permutation (shared mask across batches)
    perm = np.concatenate([np.flatnonzero(mask), np.flatnonzero(~mask)])
    ncon = int(mask.sum())
    nkeep = min(ncon, NCC)
    mask_p = np.zeros(NCC, np.float32)
    mask_p[:nkeep] = 1.0

    # mask multiplier with folded score bias exp(u^T conv_b)
    sbias = np.einsum('k,bkh->bh', conv_b, u)  # [B, NH]
    mf = mask_p.reshape(NCH, 128)  # [c, p], permuted index m = 128c + p
    maskE = np.empty((B, 128, NCH * 4), np.float32)
    for h in range(NH):
        maskE[:, :, h::4] = (mf.T[None] * np.exp(sbias)[:, None, h:h + 1])

    mht = np.concatenate([Mh[h].T for h in range(NH)], axis=1)  # [64, 256]
    mb2 = float((conv_b ** 2).mean())
    consts = dict(
        MHT=mht.astype(bf),
        C0CB=(c0 + conv_b)[:, None].astype(np.float32),
        MT64=M.T.astype(bf),
        CB4=np.repeat(conv_b[None, :], 4, 0).astype(np.float32),
        B1R=np.repeat(np.tile(b1p, 8)[None, :], 128, 0).astype(bf),
        W2R=np.repeat(np.tile(w2[0], 8)[None, :], 128, 0).astype(bf),
        I4=np.eye(4, dtype=np.float32),
        I128=np.eye(128, dtype=np.float32),
        EPSM=np.full((128, 1), 1e-5 + mb2, np.float32),
        B2C=np.full((128, 1), b2[0], np.float32),
    )
    meanb = float(conv_b.mean(dtype=np.float64))
    return ftz_halves.astype(bf), maskE.astype(bf), consts, perm, nkeep, meanb


LAST_RESULTS = None
_MEANB = None


def kernel(**inputs):
    global _BUILT, LAST_RESULTS, _MEANB
    import ml_dtypes
    from concourse.bass_utils import run_bass_kernel_spmd

    ftz_halves, maskE, consts, perm, nkeep, meanb = _host_prep(inputs)

    if _BUILT is None or _MEANB != meanb:
        # meanb is a compile-time scalar folded into an instruction immediate
        _BUILT = _build(meanb)
        _MEANB = meanb
    nc = _BUILT

    pix = np.asarray(inputs["pixel_features"], dtype=np.float32).reshape(B, C, N)
    pix_p = np.zeros((B, C, NCC), np.float32)
    pix_p[:, :, :nkeep] = pix[:, :, perm[:nkeep]]
    pixb = np.stack([pix_p[:, 0:128, :], pix_p[:, 128:256, :]], axis=1).astype(
        ml_dtypes.bfloat16)  # [B, 2, 128, NCC]

    in_maps = []
    for core in range(N_CORES):
        b0 = core * B_PER
        m = dict(consts)
        m["PIX"] = np.ascontiguousarray(pixb[b0:b0 + B_PER])
        m["FTZ"] = np.ascontiguousarray(ftz_halves[b0:b0 + B_PER])
        m["MASKE"] = np.ascontiguousarray(maskE[b0:b0 + B_PER])
        in_maps.append(m)

    res = run_bass_kernel_spmd(nc, in_maps, core_ids=list(range(N_CORES)))
    LAST_RESULTS = res
    adj_p = np.concatenate([res.results[c]["OUT"] for c in range(N_CORES)],
                           axis=0).reshape(B, NCC)
    out = np.zeros((B, N), np.float32)
    out[:, perm[:nkeep]] = adj_p[:, :nkeep]
    return out.reshape(B, H, W)
